# revision 12
# baseline (speedup 1.0000x reference)
"""Multi-head self-attention (RoPE, causal) Trainium2 Bass kernel.

Problem: B=4, S=2048, D=1024, H=16 heads, dk=64, fp32 in/out.

Sharding: 8 cores = (batch b in 0..4) x (head-group g in 0..2).
Each core computes, for its batch and its 8 heads:
  Q^T,K^T (RoPE'd), V, causal softmax attention, and the partial output
  projection  out_partial = merged_g @ WO[:, g-dims]^T  -> [2048, 1024] f32.
Host sums the two partials per batch (the "all-reduce" of the hint, done
on gather) and stacks batches.

Device layout notes:
- All matmul inputs bf16 (full PE rate), PSUM accumulation fp32.
- Q^T/K^T stored [128, 2048] per 2-head chunk; partition layout per chunk:
  [hA_even(32) | hA_odd(32) | hB_even(32) | hB_odd(32)] so RoPE's pair
  swap is a partition-block swap (done via SBUF->SBUF DMA) and the
  rotation is 3 full-width VectorE ops against host-built cos/sin tiles.
- Scores are computed transposed (S^T[k,q]) so softmax exp output E^T can
  feed P@V directly as the moving operand: U^T[d,q] = sum_k Vp[k,d]E^T[k,q]
  with Vp = [V | ones] so row 64 of U^T is the softmax denominator.
- Causal masking: per (k-tile, q-tile) only the valid q range is computed;
  the invalid prefix of E is memset to 0 and the diagonal 128x128 block is
  multiplied by a host-built triangular mask.
"""

import sys

if "/opt/trn_rl_repo" not in sys.path:
    sys.path.insert(0, "/opt/trn_rl_repo")

from contextlib import ExitStack

import numpy as np
import ml_dtypes

import concourse.bass as bass
import concourse.bacc as bacc
import concourse.tile as tile
from concourse import mybir
from concourse.bass_utils import run_bass_kernel_spmd

BF16 = ml_dtypes.bfloat16
B, S, D = 4, 2048, 1024
H, DK = 16, 64
ROPE_THETA = 10000.0
N_CORES = 8
HPC = 8          # heads per core
NCHUNK = HPC // 2  # 2-head chunks per core
NQ = S // 512    # q tiles of 512
NK = S // 128    # k tiles of 128
NI = D // 128    # contraction tiles for projections

f32 = mybir.dt.float32
bf16 = mybir.dt.bfloat16
AF = mybir.ActivationFunctionType

_CACHE = {}


def _emit(nc):
    xt = nc.declare_dram_parameter("xt", [D, S], bf16, isOutput=False)
    wqt = nc.declare_dram_parameter("wqt", [D, 512], bf16, isOutput=False)
    wkt = nc.declare_dram_parameter("wkt", [D, 512], bf16, isOutput=False)
    wvt = nc.declare_dram_parameter("wvt", [D, 512], bf16, isOutput=False)
    wot = nc.declare_dram_parameter("wot", [512, D], bf16, isOutput=False)
    cosf = nc.declare_dram_parameter("cosf", [128, S], bf16, isOutput=False)
    sinf = nc.declare_dram_parameter("sinf", [128, S], bf16, isOutput=False)
    # m4[:, :, 0:384] = 0, m4[:, a, 384+k, ... ] triangular: col 384+q holds
    # (k<=q); duplicated along axis 1 for the two packed heads.
    m4 = nc.declare_dram_parameter("m4", [128, 2, 512], bf16, isOutput=False)
    out = nc.declare_dram_parameter("out", [S, D], f32, isOutput=True)

    with tile.TileContext(nc) as tc, ExitStack() as ctx:
        consts = ctx.enter_context(tc.tile_pool(name="consts", bufs=1))
        big = ctx.enter_context(tc.tile_pool(name="big", bufs=1))
        rope = ctx.enter_context(tc.tile_pool(name="rope", bufs=2))
        epool = ctx.enter_context(tc.tile_pool(name="epool", bufs=3))
        small = ctx.enter_context(tc.tile_pool(name="small", bufs=2))
        osb_p = ctx.enter_context(tc.tile_pool(name="osbp", bufs=2))
        dram = ctx.enter_context(tc.tile_pool(name="dram", bufs=2, space="DRAM"))
        pp = ctx.enter_context(tc.tile_pool(name="pp", bufs=2, space="PSUM"))
        ps = ctx.enter_context(tc.tile_pool(name="ps", bufs=2, space="PSUM"))
        pu = ctx.enter_context(tc.tile_pool(name="pu", bufs=1, space="PSUM"))

        # ---- constant loads -------------------------------------------------
        xt_sb = consts.tile([128, NI, S], bf16, tag="xt")
        for i in range(NI):
            nc.sync.dma_start(
                out=xt_sb[:, i, :], in_=xt[128 * i : 128 * i + 128, :]
            )
        w_sb = {}
        for name, t in (("v", wvt), ("q", wqt), ("k", wkt)):
            w_sb[name] = consts.tile(
                [128, NI, 512], bf16, tag=f"w{name}", name=f"w{name}"
            )
            for i in range(NI):
                nc.sync.dma_start(
                    out=w_sb[name][:, i, :], in_=t[128 * i : 128 * i + 128, :]
                )
        wot_sb = consts.tile([128, NCHUNK, D], bf16, tag="wot")
        for t in range(NCHUNK):
            nc.sync.dma_start(
                out=wot_sb[:, t, :], in_=wot[128 * t : 128 * t + 128, :]
            )
        cos_sb = consts.tile([128, S], bf16, tag="cos")
        nc.sync.dma_start(out=cos_sb, in_=cosf[:, :])
        sin_sb = consts.tile([128, S], bf16, tag="sin")
        nc.sync.dma_start(out=sin_sb, in_=sinf[:, :])
        m4_sb = consts.tile([128, 2, 512], bf16, tag="m4")
        nc.sync.dma_start(out=m4_sb, in_=m4[:, :, :])

        # ---- persistent intermediates --------------------------------------
        v_sb = big.tile([128, NK, HPC, 65], bf16, tag="v")   # [Vp | 1]
        qrot = big.tile([128, NCHUNK, S], bf16, tag="qrot")
        krot = big.tile([128, NCHUNK, S], bf16, tag="krot")
        ut_sb = big.tile([128, NCHUNK, S], bf16, tag="ut")   # normalized U^T

        nc.vector.memset(v_sb[:, :, :, 64:65], 1.0)

        # ---- emission helpers ----------------------------------------------
        def emit_v_tile(si):
            pv = pp.tile([128, 512], f32, tag="pp", name="pv")
            for i in range(NI):
                nc.tensor.matmul(
                    pv,
                    lhsT=xt_sb[:, i, 128 * si : 128 * si + 128],
                    rhs=w_sb["v"][:, i, :],
                    start=(i == 0),
                    stop=(i == NI - 1),
                )
            nc.vector.tensor_copy(
                out=v_sb[:, si, :, 0:64],
                in_=pv.rearrange("p (h d) -> p h d", h=HPC),
            )

        qsb_tiles = {}  # (t, wname) -> accumulating sbuf tile

        def emit_proj_ss(t, wname, ss):
            if (t, wname) not in qsb_tiles:
                qsb_tiles[(t, wname)] = rope.tile(
                    [128, S], bf16, tag="qsb", name="qsb"
                )
            qsb = qsb_tiles[(t, wname)]
            pq = pp.tile([128, 512], f32, tag="pp", name="pq")
            for i in range(NI):
                nc.tensor.matmul(
                    pq,
                    lhsT=w_sb[wname][:, i, 128 * t : 128 * t + 128],
                    rhs=xt_sb[:, i, 512 * ss : 512 * ss + 512],
                    start=(i == 0),
                    stop=(i == NI - 1),
                )
            nc.vector.tensor_copy(out=qsb[:, 512 * ss : 512 * ss + 512], in_=pq)

        def emit_rope(t, wname):
            rot = qrot if wname == "q" else krot
            qsb = qsb_tiles.pop((t, wname))
            # pair-swap via partition-moving SBUF->SBUF DMAs
            qsw = rope.tile([128, S], bf16, tag="qsw")
            for dst, src in ((0, 32), (32, 0), (64, 96), (96, 64)):
                nc.sync.dma_start(
                    out=qsw[dst : dst + 32, :], in_=qsb[src : src + 32, :]
                )
            nc.vector.tensor_mul(qsw, qsw, sin_sb)
            nc.vector.tensor_mul(qsb, qsb, cos_sb)
            nc.vector.tensor_add(rot[:, t, :], qsb, qsw)

        def emit_attn(t, j):
            nk = 4 * j + 4
            pua = pu.tile([65, 512], f32, tag="pua", name="pua")
            pub = pu.tile([65, 512], f32, tag="pub", name="pub")
            for i in range(nk):
                qr0 = max(0, 128 * (i - 4 * j))
                ps_t = ps.tile([128, 2, 512], f32, tag="ps", name="ps_t")
                for a in (0, 1):
                    nc.tensor.matmul(
                        ps_t[:, a, qr0:512],
                        lhsT=krot[64 * a : 64 * a + 64, t, 128 * i : 128 * i + 128],
                        rhs=qrot[
                            64 * a : 64 * a + 64, t, 512 * j + qr0 : 512 * j + 512
                        ],
                        start=True,
                        stop=True,
                    )
                e_t = epool.tile([128, 2, 512], bf16, tag="e", name="e_t")
                nc.scalar.activation(
                    out=e_t[:, :, qr0:512],
                    in_=ps_t[:, :, qr0:512],
                    func=AF.Exp,
                    scale=0.125,
                )
                if qr0 > 0:
                    nc.vector.memset(e_t[:, :, 0:qr0], 0.0)
                if i >= 4 * j:  # diagonal block: triangular mask
                    sd = i - 4 * j
                    nc.vector.tensor_mul(
                        e_t[:, :, 128 * sd : 128 * sd + 128],
                        e_t[:, :, 128 * sd : 128 * sd + 128],
                        m4_sb[:, :, 384:512],
                    )
                for a, pab in ((0, pua), (1, pub)):
                    nc.tensor.matmul(
                        pab[:, qr0:512],
                        lhsT=v_sb[:, i, 2 * t + a, :],
                        rhs=e_t[:, a, qr0:512],
                        start=(i == 0),
                        stop=(i == nk - 1),
                    )
            # normalization: ut = U / rowsum  (rowsum in row 64)
            rb = small.tile([65, 2, 512], f32, tag="rb", name="rb")
            nc.vector.reciprocal(rb[64:65, 0, :], pua[64:65, :])
            nc.vector.reciprocal(rb[64:65, 1, :], pub[64:65, :])
            nc.gpsimd.partition_broadcast(rb[0:64, :, :], rb[64:65, :, :])
            nc.vector.tensor_mul(
                ut_sb[0:64, t, 512 * j : 512 * j + 512], pua[0:64, :], rb[0:64, 0, :]
            )
            utb = small.tile([64, 512], bf16, tag="utb", name="utb")
            nc.vector.tensor_mul(utb, pub[0:64, :], rb[0:64, 1, :])
            nc.sync.dma_start(
                out=ut_sb[64:128, t, 512 * j : 512 * j + 512], in_=utb
            )

        def emit_outproj(qi, ch):
            po = pp.tile([128, 512], f32, tag="pp", name="po")
            for t in range(NCHUNK):
                nc.tensor.matmul(
                    po,
                    lhsT=ut_sb[:, t, 128 * qi : 128 * qi + 128],
                    rhs=wot_sb[:, t, 512 * ch : 512 * ch + 512],
                    start=(t == 0),
                    stop=(t == NCHUNK - 1),
                )
            osb = osb_p.tile([128, 512], f32, tag="osb", name="osb")
            nc.vector.tensor_copy(out=osb, in_=po)
            nc.sync.dma_start(
                out=out[128 * qi : 128 * qi + 128, 512 * ch : 512 * ch + 512],
                in_=osb,
            )

        # ---- emission order: keep PE fed with projections while ACT does
        # exps; attention chunk t overlaps projections of chunk t+1 ---------
        for si in range(8):
            emit_v_tile(si)
        for wname in ("q", "k"):
            for ss in range(NQ):
                emit_proj_ss(0, wname, ss)
            emit_rope(0, wname)
        for si in range(8, NK):
            emit_v_tile(si)
        for t in range(NCHUNK):
            # proj pieces of chunk t+1 interleaved at attention j granularity
            for j in range(NQ):
                emit_attn(t, j)
                if t + 1 < NCHUNK:
                    wname = "q" if j < 2 else "k"
                    emit_proj_ss(t + 1, wname, 2 * (j % 2))
                    emit_proj_ss(t + 1, wname, 2 * (j % 2) + 1)
                    if j % 2 == 1:
                        emit_rope(t + 1, wname)
                else:
                    # last chunk: fill PE with output projection for the
                    # q-tiles whose ut rows are now complete
                    for qi in range(4 * j, 4 * j + 4):
                        for ch in range(2):
                            emit_outproj(qi, ch)
    return nc


def _build():
    if "nc" not in _CACHE:
        nc = bacc.Bacc(
            "TRN2",
            target_bir_lowering=False,
            debug=False,
            num_devices=N_CORES,
        )
        nc = _emit(nc)
        nc.compile()
        _CACHE["nc"] = nc
    return _CACHE["nc"]


def _host_inputs(x, WQ, WK, WV, WO):
    """Build the 8 per-core input dicts."""
    # RoPE permutation for Q/K weight rows, per 2-head chunk layout:
    # chunk col p: [hA evens(32) | hA odds(32) | hB evens(32) | hB odds(32)]
    perm = np.empty(512, dtype=np.int64)
    for jj in range(512):
        t, r = divmod(jj, 128)
        ab, rr = divmod(r, 64)
        eo, i = divmod(rr, 32)
        h_loc = 2 * t + ab
        perm[jj] = h_loc * 64 + 2 * i + eo

    pos = np.arange(S, dtype=np.float64)[:, None]
    dims = np.arange(DK // 2, dtype=np.float64)[None, :]
    thetas = pos / (ROPE_THETA ** (2.0 * dims / DK))  # [S, 32]
    cos = np.cos(thetas).T  # [32, S]
    sin = np.sin(thetas).T
    cosf = np.tile(cos, (4, 1)).astype(BF16)  # [128, S]
    sinf = np.concatenate([-sin, sin, -sin, sin], axis=0).astype(BF16)

    m4 = np.zeros((128, 2, 512), dtype=BF16)
    tri = (np.arange(128)[:, None] <= np.arange(128)[None, :]).astype(BF16)
    m4[:, 0, 384:512] = tri
    m4[:, 1, 384:512] = tri

    xt_all = [np.ascontiguousarray(x[b].T).astype(BF16) for b in range(B)]
    in_maps = []
    for c in range(N_CORES):
        b, g = divmod(c, 2)
        rows = g * 512 + np.arange(512)
        in_maps.append(
            {
                "xt": xt_all[b],
                "wqt": np.ascontiguousarray(WQ[rows[perm], :].T).astype(BF16),
                "wkt": np.ascontiguousarray(WK[rows[perm], :].T).astype(BF16),
                "wvt": np.ascontiguousarray(WV[rows, :].T).astype(BF16),
                "wot": np.ascontiguousarray(WO[:, rows].T).astype(BF16),
                "cosf": cosf,
                "sinf": sinf,
                "m4": m4,
            }
        )
    return in_maps


def kernel(x, WQ, WK, WV, WO):
    nc = _build()
    in_maps = _host_inputs(
        np.asarray(x), np.asarray(WQ), np.asarray(WK), np.asarray(WV), np.asarray(WO)
    )
    res = run_bass_kernel_spmd(nc, in_maps, list(range(N_CORES))).results
    out = np.empty((B, S, D), dtype=np.float32)
    for b in range(B):
        out[b] = np.asarray(res[2 * b]["out"]) + np.asarray(res[2 * b + 1]["out"])
    return out


# revision 16
# speedup vs baseline: 156.7613x; 156.7613x over previous
"""Multi-head self-attention (RoPE, causal) Trainium2 Bass kernel.

Problem: B=4, S=2048, D=1024, H=16 heads, dk=64, fp32 in/out.

Sharding: 8 cores = (batch b in 0..4) x (head-group g in 0..2).
Each core computes, for its batch and its 8 heads:
  Q^T,K^T (RoPE'd), V, causal softmax attention, and the partial output
  projection  out_partial = merged_g @ WO[:, g-dims]^T  -> [2048, 1024] f32.
Host sums the two partials per batch (the "all-reduce" of the hint, done
on gather) and stacks batches.

Device layout notes:
- All matmul inputs bf16 (full PE rate), PSUM accumulation fp32.
- Q^T/K^T stored [128, 2048] per 2-head chunk; partition layout per chunk:
  [hA_even(32) | hA_odd(32) | hB_even(32) | hB_odd(32)] so RoPE's pair
  swap is a partition-block swap (done via SBUF->SBUF DMA) and the
  rotation is 3 full-width VectorE ops against host-built cos/sin tiles.
- Scores are computed transposed (S^T[k,q]) so softmax exp output E^T can
  feed P@V directly as the moving operand: U^T[d,q] = sum_k Vp[k,d]E^T[k,q]
  with Vp = [V | ones] so row 64 of U^T is the softmax denominator.
- Causal masking: per (k-tile, q-tile) only the valid q range is computed;
  the invalid prefix of E is memset to 0 and the diagonal 128x128 block is
  multiplied by a host-built triangular mask.
"""

import sys

if "/opt/trn_rl_repo" not in sys.path:
    sys.path.insert(0, "/opt/trn_rl_repo")

from contextlib import ExitStack

import numpy as np
import ml_dtypes

import concourse.bass as bass
import concourse.bacc as bacc
import concourse.tile as tile
from concourse import mybir
from concourse.bass_utils import run_bass_kernel_spmd

BF16 = ml_dtypes.bfloat16
B, S, D = 4, 2048, 1024
H, DK = 16, 64
ROPE_THETA = 10000.0
N_CORES = 8
HPC = 8          # heads per core
NCHUNK = HPC // 2  # 2-head chunks per core
NQ = S // 512    # q tiles of 512
NK = S // 128    # k tiles of 128
NI = D // 128    # contraction tiles for projections

f32 = mybir.dt.float32
bf16 = mybir.dt.bfloat16
AF = mybir.ActivationFunctionType

_CACHE = {}


def _emit(nc, reps=1):
    xt = nc.declare_dram_parameter("xt", [D, S], bf16, isOutput=False)
    wqt = nc.declare_dram_parameter("wqt", [D, 512], bf16, isOutput=False)
    wkt = nc.declare_dram_parameter("wkt", [D, 512], bf16, isOutput=False)
    wvt = nc.declare_dram_parameter("wvt", [D, 512], bf16, isOutput=False)
    wot = nc.declare_dram_parameter("wot", [512, D], bf16, isOutput=False)
    cosf = nc.declare_dram_parameter("cosf", [128, S], bf16, isOutput=False)
    sinf = nc.declare_dram_parameter("sinf", [128, S], bf16, isOutput=False)
    # m4[:, :, 0:384] = 0, m4[:, a, 384+k, ... ] triangular: col 384+q holds
    # (k<=q); duplicated along axis 1 for the two packed heads.
    m4 = nc.declare_dram_parameter("m4", [128, 2, 512], bf16, isOutput=False)
    out = nc.declare_dram_parameter("out", [S, D], f32, isOutput=True)

    with tile.TileContext(nc) as tc, ExitStack() as ctx:
        consts = ctx.enter_context(tc.tile_pool(name="consts", bufs=1))
        big = ctx.enter_context(tc.tile_pool(name="big", bufs=1))
        rope = ctx.enter_context(tc.tile_pool(name="rope", bufs=2))
        epool = ctx.enter_context(tc.tile_pool(name="epool", bufs=3))
        small = ctx.enter_context(tc.tile_pool(name="small", bufs=2))
        osb_p = ctx.enter_context(tc.tile_pool(name="osbp", bufs=2))
        dram = ctx.enter_context(tc.tile_pool(name="dram", bufs=2, space="DRAM"))
        pp = ctx.enter_context(tc.tile_pool(name="pp", bufs=2, space="PSUM"))
        ps = ctx.enter_context(tc.tile_pool(name="ps", bufs=2, space="PSUM"))
        pu = ctx.enter_context(tc.tile_pool(name="pu", bufs=1, space="PSUM"))

        # ---- constant loads -------------------------------------------------
        xt_sb = consts.tile([128, NI, S], bf16, tag="xt")
        for i in range(NI):
            nc.sync.dma_start(
                out=xt_sb[:, i, :], in_=xt[128 * i : 128 * i + 128, :]
            )
        w_sb = {}
        for name, t in (("v", wvt), ("q", wqt), ("k", wkt)):
            w_sb[name] = consts.tile(
                [128, NI, 512], bf16, tag=f"w{name}", name=f"w{name}"
            )
            for i in range(NI):
                nc.sync.dma_start(
                    out=w_sb[name][:, i, :], in_=t[128 * i : 128 * i + 128, :]
                )
        wot_sb = consts.tile([128, NCHUNK, D], bf16, tag="wot")
        for t in range(NCHUNK):
            nc.sync.dma_start(
                out=wot_sb[:, t, :], in_=wot[128 * t : 128 * t + 128, :]
            )
        cos_sb = consts.tile([128, S], bf16, tag="cos")
        nc.sync.dma_start(out=cos_sb, in_=cosf[:, :])
        sin_sb = consts.tile([128, S], bf16, tag="sin")
        nc.sync.dma_start(out=sin_sb, in_=sinf[:, :])
        m4_sb = consts.tile([128, 2, 512], bf16, tag="m4")
        nc.sync.dma_start(out=m4_sb, in_=m4[:, :, :])

        # ---- persistent intermediates --------------------------------------
        v_sb = big.tile([128, NK, HPC, 65], bf16, tag="v")   # [Vp | 1]
        qrot = big.tile([128, NCHUNK, S], bf16, tag="qrot")
        krot = big.tile([128, NCHUNK, S], bf16, tag="krot")
        ut_sb = big.tile([128, NCHUNK, S], bf16, tag="ut")   # normalized U^T

        # ---- emission helpers ----------------------------------------------
        def emit_v_tile(si):
            pv = pp.tile([128, 512], f32, tag="pp", name="pv")
            for i in range(NI):
                nc.tensor.matmul(
                    pv,
                    lhsT=xt_sb[:, i, 128 * si : 128 * si + 128],
                    rhs=w_sb["v"][:, i, :],
                    start=(i == 0),
                    stop=(i == NI - 1),
                )
            nc.vector.tensor_copy(
                out=v_sb[:, si, :, 0:64],
                in_=pv.rearrange("p (h d) -> p h d", h=HPC),
            )

        qsb_tiles = {}  # (t, wname) -> accumulating sbuf tile

        def emit_proj_ss(t, wname, ss):
            if (t, wname) not in qsb_tiles:
                qsb_tiles[(t, wname)] = rope.tile(
                    [128, S], bf16, tag="qsb", name="qsb"
                )
            qsb = qsb_tiles[(t, wname)]
            pq = pp.tile([128, 512], f32, tag="pp", name="pq")
            for i in range(NI):
                nc.tensor.matmul(
                    pq,
                    lhsT=w_sb[wname][:, i, 128 * t : 128 * t + 128],
                    rhs=xt_sb[:, i, 512 * ss : 512 * ss + 512],
                    start=(i == 0),
                    stop=(i == NI - 1),
                )
            nc.vector.tensor_copy(out=qsb[:, 512 * ss : 512 * ss + 512], in_=pq)

        def emit_rope(t, wname):
            rot = qrot if wname == "q" else krot
            qsb = qsb_tiles.pop((t, wname))
            # pair-swap via partition-moving SBUF->SBUF DMAs
            qsw = rope.tile([128, S], bf16, tag="qsw")
            for dst, src in ((0, 32), (32, 0), (64, 96), (96, 64)):
                nc.sync.dma_start(
                    out=qsw[dst : dst + 32, :], in_=qsb[src : src + 32, :]
                )
            nc.vector.tensor_mul(qsw, qsw, sin_sb)
            nc.vector.tensor_mul(qsb, qsb, cos_sb)
            nc.vector.tensor_add(rot[:, t, :], qsb, qsw)

        def emit_attn(t, j):
            nk = 4 * j + 4
            pua = pu.tile([65, 512], f32, tag="pua", name="pua")
            pub = pu.tile([65, 512], f32, tag="pub", name="pub")
            for i in range(nk):
                qr0 = max(0, 128 * (i - 4 * j))
                ps_t = ps.tile([128, 2, 512], f32, tag="ps", name="ps_t")
                for a in (0, 1):
                    nc.tensor.matmul(
                        ps_t[:, a, qr0:512],
                        lhsT=krot[64 * a : 64 * a + 64, t, 128 * i : 128 * i + 128],
                        rhs=qrot[
                            64 * a : 64 * a + 64, t, 512 * j + qr0 : 512 * j + 512
                        ],
                        start=True,
                        stop=True,
                    )
                e_t = epool.tile([128, 2, 512], bf16, tag="e", name="e_t")
                nc.scalar.activation(
                    out=e_t[:, :, qr0:512],
                    in_=ps_t[:, :, qr0:512],
                    func=AF.Exp,
                    scale=0.125,
                )
                if qr0 > 0:
                    nc.vector.memset(e_t[:, :, 0:qr0], 0.0)
                if i >= 4 * j:  # diagonal block: triangular mask
                    sd = i - 4 * j
                    nc.vector.tensor_mul(
                        e_t[:, :, 128 * sd : 128 * sd + 128],
                        e_t[:, :, 128 * sd : 128 * sd + 128],
                        m4_sb[:, :, 384:512],
                    )
                for a, pab in ((0, pua), (1, pub)):
                    nc.tensor.matmul(
                        pab[:, qr0:512],
                        lhsT=v_sb[:, i, 2 * t + a, :],
                        rhs=e_t[:, a, qr0:512],
                        start=(i == 0),
                        stop=(i == nk - 1),
                    )
            # normalization: ut = U / rowsum  (rowsum in row 64)
            rb = small.tile([65, 2, 512], f32, tag="rb", name="rb")
            nc.vector.reciprocal(rb[64:65, 0, :], pua[64:65, :])
            nc.vector.reciprocal(rb[64:65, 1, :], pub[64:65, :])
            nc.gpsimd.partition_broadcast(rb[0:64, :, :], rb[64:65, :, :])
            nc.vector.tensor_mul(
                ut_sb[0:64, t, 512 * j : 512 * j + 512], pua[0:64, :], rb[0:64, 0, :]
            )
            utb = small.tile([64, 512], bf16, tag="utb", name="utb")
            nc.vector.tensor_mul(utb, pub[0:64, :], rb[0:64, 1, :])
            nc.sync.dma_start(
                out=ut_sb[64:128, t, 512 * j : 512 * j + 512], in_=utb
            )

        def emit_outproj(qi, ch):
            po = pp.tile([128, 512], f32, tag="pp", name="po")
            for t in range(NCHUNK):
                nc.tensor.matmul(
                    po,
                    lhsT=ut_sb[:, t, 128 * qi : 128 * qi + 128],
                    rhs=wot_sb[:, t, 512 * ch : 512 * ch + 512],
                    start=(t == 0),
                    stop=(t == NCHUNK - 1),
                )
            osb = osb_p.tile([128, 512], f32, tag="osb", name="osb")
            nc.vector.tensor_copy(out=osb, in_=po)
            nc.sync.dma_start(
                out=out[128 * qi : 128 * qi + 128, 512 * ch : 512 * ch + 512],
                in_=osb,
            )

        # ---- emission order: keep PE fed with projections while ACT does
        # exps; attention chunk t overlaps projections of chunk t+1 ---------
        def emit_body():
            nc.vector.memset(v_sb[:, :, :, 64:65], 1.0)
            for si in range(8):
                emit_v_tile(si)
            for wname in ("q", "k"):
                for ss in range(NQ):
                    emit_proj_ss(0, wname, ss)
                emit_rope(0, wname)
            for si in range(8, NK):
                emit_v_tile(si)
            for t in range(NCHUNK):
                # chunk t+1 projections interleaved at attention j granularity
                for j in range(NQ):
                    emit_attn(t, j)
                    if t + 1 < NCHUNK:
                        wname = "q" if j < 2 else "k"
                        emit_proj_ss(t + 1, wname, 2 * (j % 2))
                        emit_proj_ss(t + 1, wname, 2 * (j % 2) + 1)
                        if j % 2 == 1:
                            emit_rope(t + 1, wname)
                    else:
                        # last chunk: fill PE with output projection for the
                        # q-tiles whose ut rows are now complete
                        for qi in range(4 * j, 4 * j + 4):
                            for ch in range(2):
                                emit_outproj(qi, ch)

        for _rep in range(reps):
            emit_body()
    return nc


def _build():
    if "nc" not in _CACHE:
        nc = bacc.Bacc(
            "TRN2",
            target_bir_lowering=False,
            debug=False,
            num_devices=N_CORES,
        )
        nc = _emit(nc)
        nc.compile()
        _CACHE["nc"] = nc
    return _CACHE["nc"]


def _host_inputs(x, WQ, WK, WV, WO):
    """Build the 8 per-core input dicts."""
    # RoPE permutation for Q/K weight rows, per 2-head chunk layout:
    # chunk col p: [hA evens(32) | hA odds(32) | hB evens(32) | hB odds(32)]
    perm = np.empty(512, dtype=np.int64)
    for jj in range(512):
        t, r = divmod(jj, 128)
        ab, rr = divmod(r, 64)
        eo, i = divmod(rr, 32)
        h_loc = 2 * t + ab
        perm[jj] = h_loc * 64 + 2 * i + eo

    pos = np.arange(S, dtype=np.float64)[:, None]
    dims = np.arange(DK // 2, dtype=np.float64)[None, :]
    thetas = pos / (ROPE_THETA ** (2.0 * dims / DK))  # [S, 32]
    cos = np.cos(thetas).T  # [32, S]
    sin = np.sin(thetas).T
    cosf = np.tile(cos, (4, 1)).astype(BF16)  # [128, S]
    sinf = np.concatenate([-sin, sin, -sin, sin], axis=0).astype(BF16)

    m4 = np.zeros((128, 2, 512), dtype=BF16)
    tri = (np.arange(128)[:, None] <= np.arange(128)[None, :]).astype(BF16)
    m4[:, 0, 384:512] = tri
    m4[:, 1, 384:512] = tri

    xt_all = [np.ascontiguousarray(x[b].T).astype(BF16) for b in range(B)]
    in_maps = []
    for c in range(N_CORES):
        b, g = divmod(c, 2)
        rows = g * 512 + np.arange(512)
        in_maps.append(
            {
                "xt": xt_all[b],
                "wqt": np.ascontiguousarray(WQ[rows[perm], :].T).astype(BF16),
                "wkt": np.ascontiguousarray(WK[rows[perm], :].T).astype(BF16),
                "wvt": np.ascontiguousarray(WV[rows, :].T).astype(BF16),
                "wot": np.ascontiguousarray(WO[:, rows].T).astype(BF16),
                "cosf": cosf,
                "sinf": sinf,
                "m4": m4,
            }
        )
    return in_maps


def kernel(x, WQ, WK, WV, WO):
    nc = _build()
    in_maps = _host_inputs(
        np.asarray(x), np.asarray(WQ), np.asarray(WK), np.asarray(WV), np.asarray(WO)
    )
    res = run_bass_kernel_spmd(nc, in_maps, list(range(N_CORES))).results
    out = np.empty((B, S, D), dtype=np.float32)
    for b in range(B):
        out[b] = np.asarray(res[2 * b]["out"]) + np.asarray(res[2 * b + 1]["out"])
    return out
